# revision 1
# baseline (speedup 1.0000x reference)
"""TRN2 Bass kernel v3 for CrossOpLayerUTPM — batch-sharded, (i,d)-expanded.

out[b,(i,j)] = x[b,i] x[b,j] s[i,j].  Diagonals d=j-i in 8 chunks of 32.
Column layout m = OFFC[c] + i*32 + dd  (d = 1+32c+dd).

Per (chunk, batch-tile) two plain-2D bf16 DVE ops (all partition-base 0,
all contiguous free ranges — the only fast DVE paths on cayman):
    t1 = xE[:, 32*d0 : 32*(d0+W)] * xR[:, 0:32W]     # x[b,i+d] * x[b,i]
    o  = t1 * S[:, chunk]                             # * s[i,i+d]
xE[b, m*32+t] = x[b, m+t] (host-built sliding window), xR[b, i*32+dd] =
x[b, i] (host-built repeat), S broadcast rows (host).  bf16 out; host
drops garbage columns (j>255) and reorders pairs.
"""
import numpy as np
import ml_dtypes
from contextlib import ExitStack

import jax
from jax.sharding import Mesh, PartitionSpec
from jax.experimental.shard_map import shard_map

import concourse.bass as bass
import concourse.bacc as bacc
import concourse.tile as tile
from concourse import mybir
from concourse.bass2jax import (
    _bass_exec_p,
    install_neuronx_cc_hook,
    partition_id_tensor,
)

F32 = mybir.dt.float32
BF16 = mybir.dt.bfloat16
BF16NP = ml_dtypes.bfloat16

B, NCOL = 4096, 256
NCORES = 8
BPC = B // NCORES        # 512
NT = BPC // 128          # 4
NCH = 8
D0 = [1 + 32 * c for c in range(NCH)]
WC = [NCOL - d0 for d0 in D0]
OFFC = np.concatenate([[0], np.cumsum([32 * w for w in WC])]).astype(np.int64)
TOTF = int(OFFC[-1])     # 36608
XE_LEN = NCOL * 32       # 8192


def _build_nc(reps=1):
    nc = bacc.Bacc("TRN2", target_bir_lowering=False, debug=False)
    xe_in = nc.dram_tensor("xe", [BPC, XE_LEN], BF16, kind="ExternalInput")
    xr_in = nc.dram_tensor("xr", [BPC, XE_LEN], BF16, kind="ExternalInput")
    s_in = nc.dram_tensor("sb", [128, TOTF], BF16, kind="ExternalInput")
    out_t = nc.dram_tensor("out", [BPC, TOTF], BF16, kind="ExternalOutput")

    with tile.TileContext(nc) as tc, ExitStack() as ctx:
        cpool = ctx.enter_context(tc.tile_pool(name="const", bufs=1))
        xpool = ctx.enter_context(tc.tile_pool(name="xtiles", bufs=2))
        work = ctx.enter_context(tc.tile_pool(name="work", bufs=2))

        s_all = cpool.tile([128, TOTF], BF16, name="s_all")
        nc.sync.dma_start(out=s_all[:, :], in_=s_in[:, :])

        for r in range(reps):
          for t in range(NT):
            xe = xpool.tile([128, XE_LEN], BF16, tag="xe", name=f"xe{r}_{t}",
                            bufs=2)
            nc.sync.dma_start(out=xe[:, :], in_=xe_in[t * 128:(t + 1) * 128, :])
            xr = xpool.tile([128, XE_LEN], BF16, tag="xr", name=f"xr{r}_{t}",
                            bufs=2)
            nc.sync.dma_start(out=xr[:, :], in_=xr_in[t * 128:(t + 1) * 128, :])
            for c in range(NCH):
                d0, w = D0[c], WC[c]
                for h in range(2):
                    wlo = (w // 2) * h
                    whi = w if h else (w // 2)
                    fsz = 32 * (whi - wlo)
                    off = int(OFFC[c]) + 32 * wlo
                    exo = 32 * (d0 + wlo)
                    t1 = work.tile([128, 4096], BF16, tag="t1",
                                   name=f"t{r}_{c}_{t}_{h}", bufs=3)
                    nc.vector.tensor_mul(
                        t1[:, 0:fsz], xe[:, exo:exo + fsz],
                        xr[:, 32 * wlo:32 * wlo + fsz])
                    o = work.tile([128, 4096], BF16, tag="o",
                                  name=f"o{r}_{c}_{t}_{h}", bufs=3)
                    nc.vector.tensor_mul(o[:, 0:fsz], t1[:, 0:fsz],
                                         s_all[:, off:off + fsz])
                    nc.sync.dma_start(
                        out=out_t[t * 128:(t + 1) * 128, off:off + fsz],
                        in_=o[:, 0:fsz])

    nc.compile()
    return nc


class _Runner:
    def __init__(self, nc, n_cores=NCORES):
        install_neuronx_cc_hook()
        self.nc = nc
        self.n_cores = n_cores
        partition_name = (
            nc.partition_id_tensor.name if nc.partition_id_tensor else None
        )
        in_names, out_names, out_avals, zero_outs = [], [], [], []
        for alloc in nc.m.functions[0].allocations:
            if not isinstance(alloc, mybir.MemoryLocationSet):
                continue
            name = alloc.memorylocations[0].name
            if alloc.kind == "ExternalInput":
                if name != partition_name:
                    in_names.append(name)
            elif alloc.kind == "ExternalOutput":
                shape = tuple(alloc.tensor_shape)
                dtype = mybir.dt.np(alloc.dtype)
                out_avals.append(jax.core.ShapedArray(shape, dtype))
                zero_outs.append(np.zeros(shape, dtype))
                out_names.append(name)
        self.n_params = len(in_names)
        self.param_names = list(in_names)
        self.out_names = out_names
        self.out_avals = out_avals
        self.zero_outs = zero_outs
        all_in = in_names + out_names
        if partition_name is not None:
            all_in.append(partition_name)

        def _body(*args):
            operands = list(args)
            if partition_name is not None:
                operands.append(partition_id_tensor())
            return tuple(_bass_exec_p.bind(
                *operands,
                out_avals=tuple(out_avals),
                in_names=tuple(all_in),
                out_names=tuple(out_names),
                lowering_input_output_aliases=(),
                sim_require_finite=False,
                sim_require_nnan=False,
                nc=nc,
            ))

        devices = jax.devices()[:n_cores]
        mesh = Mesh(np.asarray(devices), ("core",))
        n_outs = len(out_names)
        in_specs = (PartitionSpec("core"),) * (self.n_params + n_outs)
        out_specs = (PartitionSpec("core"),) * n_outs
        self.fn = jax.jit(
            shard_map(_body, mesh=mesh, in_specs=in_specs,
                      out_specs=out_specs, check_rep=False),
            keep_unused=True,
        )

    def run_concat(self, concat_in):
        concat_zeros = [
            np.zeros((self.n_cores * z.shape[0], *z.shape[1:]), z.dtype)
            for z in self.zero_outs
        ]
        outs = self.fn(*concat_in, *concat_zeros)
        return [np.asarray(o) for o in outs]


_CACHE = {}


def _get_runner(reps=1):
    if reps not in _CACHE:
        _CACHE[reps] = _Runner(_build_nc(reps))
    return _CACHE[reps]


def _host_prep(x, latent_emb):
    x = np.asarray(x, np.float32)
    L = np.asarray(latent_emb, np.float32)
    s = (L @ L.T).astype(np.float32)

    # S_flat[(c, i, dd)] = s[i, i + 1 + 32c + dd] (0 where j > 255)
    s_flat = np.zeros(TOTF, np.float32)
    for c in range(NCH):
        d0, w = D0[c], WC[c]
        ii, dd = np.meshgrid(np.arange(w), np.arange(32), indexing="ij")
        j = ii + d0 + dd
        blk = np.zeros((w, 32), np.float32)
        valid = j <= NCOL - 1
        blk[valid] = s[ii[valid], j[valid]]
        s_flat[OFFC[c]:OFFC[c + 1]] = blk.reshape(-1)
    s_bcast = np.broadcast_to(s_flat.astype(BF16NP), (128, TOTF)).copy()

    # per-core xE (sliding windows) and xR (32x repeat)
    xb = x.astype(BF16NP)
    xpad = np.zeros((B, NCOL + 32), BF16NP)
    xpad[:, :NCOL] = xb
    win = np.lib.stride_tricks.sliding_window_view(
        xpad, 32, axis=1)[:, :NCOL, :]                    # [B, 256, 32]
    xE = win.reshape(B, XE_LEN)
    xR = np.repeat(xb, 32, axis=1)                        # [B, 8192]

    xe_cores = [np.ascontiguousarray(xE[c * BPC:(c + 1) * BPC])
                for c in range(NCORES)]
    xr_cores = [np.ascontiguousarray(xR[c * BPC:(c + 1) * BPC])
                for c in range(NCORES)]
    return xe_cores, xr_cores, s_bcast


_IDX = None


def _pair_index():
    global _IDX
    if _IDX is None:
        iu, ju = np.triu_indices(NCOL, k=1)
        d = ju - iu
        c = (d - 1) // 32
        dd = d - 1 - 32 * c
        _IDX = (OFFC[c] + iu * 32 + dd).astype(np.int64)
    return _IDX


def kernel(x, latent_emb):
    xe_cores, xr_cores, s_bcast = _host_prep(x, latent_emb)
    runner = _get_runner()
    concat_in = []
    for name in runner.param_names:
        if name == "xe":
            concat_in.append(np.concatenate(xe_cores, axis=0))
        elif name == "xr":
            concat_in.append(np.concatenate(xr_cores, axis=0))
        elif name == "sb":
            concat_in.append(np.concatenate([s_bcast] * NCORES, axis=0))
        else:
            raise KeyError(name)
    outs = runner.run_concat(concat_in)
    dev = outs[runner.out_names.index("out")]     # [4096, TOTF] bf16
    return dev[:, _pair_index()].astype(np.float32)



# revision 5
# speedup vs baseline: 5.9373x; 5.9373x over previous
"""TRN2 Bass kernel v4 for CrossOpLayerUTPM — wrapped-diagonal layout.

out[b,(i,j)] = x[b,i] x[b,j] s[i,j] for strict upper pairs of N=256.

Wrapped diagonals: for d=1..127, column block (d-1) holds
  out_w[b, (d-1)*256 + i] = x[b,i] * x[b,(i+d)%256] * s[i,(i+d)%256]
covering true diagonals {d, 256-d} exactly once; block 127's first 128
columns hold diagonal 128. Total P = 127*256 + 128 = 32640 — no padding.

Per 128-row batch tile, pass 1 is 3 bf16 DVE tensor_mul ops with 3D APs
reading straight from a doubled x copy in SBUF (stride-0 d-dim for the
x_i factor, stride-2 d-dim over even/odd-shifted windows for x_j).
Pass 2 multiplies by the SBUF-resident S table in contiguous chunks,
each DMA'd out as it completes. Only x (256KB/core) is read per rep;
output 33.4MB/core bf16 written — near the HBM write roofline.
Host reorders wrapped columns to triu pair order at the end.
"""
import os
import numpy as np
import ml_dtypes
from contextlib import ExitStack

import jax
from jax.sharding import Mesh, PartitionSpec
from jax.experimental.shard_map import shard_map

import concourse.bass as bass
import concourse.bacc as bacc
import concourse.tile as tile
from concourse import mybir
from concourse.bass_types import AP
from concourse.bass2jax import (
    _bass_exec_p,
    install_neuronx_cc_hook,
    partition_id_tensor,
)

F32 = mybir.dt.float32
BF16 = mybir.dt.bfloat16
BF16NP = ml_dtypes.bfloat16

B, NCOL = 4096, 256
NCORES = 8
BPC = B // NCORES        # 512
NT = BPC // 128          # 4
P = 127 * NCOL + 128     # 32640
CHUNKS = 4
CH = P // CHUNKS         # 8160
VARIANT = os.environ.get("K4_VARIANT", "b3d")   # "b3d" | "per_d"
# pass-2 chunks (out of 8) routed to GPSIMD instead of DVE, encoded in the
# variant string as e.g. "b3d_g3" (3 of 8 chunks on gpsimd).


def _win3d(full_ap, nblk, blk_stride, inner, offset):
    """[128, nblk, inner] AP over `full_ap`'s tensor: block stride
    `blk_stride`, unit inner stride, element `offset` base."""
    p = full_ap.ap[0]
    return AP(tensor=full_ap.tensor,
              ap=[[int(p[0]), int(p[1])], [blk_stride, nblk], [1, inner]],
              offset=offset)


def _pass1_b3d(nc, t1, xx, xxo):
    """3 tensor_mul ops: even-d batch, odd-d batch, d=128 half row."""
    xx_t = xx[:, :]
    xxo_t = xxo[:, :]
    t1_t = t1[:, :]
    # even d = 2,4,...,126 (63 blocks, out block index d-1 odd)
    in1 = xx[:, 0:NCOL].unsqueeze(1).broadcast_to([128, 63, NCOL])
    in2 = _win3d(xx_t, 63, 2, NCOL, 2)
    out = _win3d(t1_t, 63, 2 * NCOL, NCOL, NCOL)
    nc.vector.tensor_mul(out, in1, in2)
    # odd d = 1,3,...,127 (64 blocks, out block index d-1 even)
    in1 = xx[:, 0:NCOL].unsqueeze(1).broadcast_to([128, 64, NCOL])
    in2 = _win3d(xxo_t, 64, 2, NCOL, 0)
    out = _win3d(t1_t, 64, 2 * NCOL, NCOL, 0)
    nc.vector.tensor_mul(out, in1, in2)
    # d = 128 half row
    nc.vector.tensor_mul(t1[:, 127 * NCOL:P], xx[:, 0:128], xx[:, 128:256])


def _pass1_per_d(nc, t1, xx, xxo):
    """Fallback: one 2D tensor_mul per wrapped diagonal."""
    for d in range(1, 128):
        nc.vector.tensor_mul(
            t1[:, (d - 1) * NCOL:d * NCOL], xx[:, 0:NCOL], xx[:, d:d + NCOL])
    nc.vector.tensor_mul(t1[:, 127 * NCOL:P], xx[:, 0:128], xx[:, 128:256])


def _build_nc(reps=1, variant=VARIANT):
    nc = bacc.Bacc("TRN2", target_bir_lowering=False, debug=False)
    x_in = nc.dram_tensor("x", [BPC, NCOL], BF16, kind="ExternalInput")
    s_in = nc.dram_tensor("sw", [128, P], BF16, kind="ExternalInput")
    out_t = nc.dram_tensor("out", [BPC, P], BF16, kind="ExternalOutput")

    with tile.TileContext(nc) as tc, ExitStack() as ctx:
        cpool = ctx.enter_context(tc.tile_pool(name="const", bufs=1))
        xpool = ctx.enter_context(tc.tile_pool(name="xtiles", bufs=2))
        tpool = ctx.enter_context(tc.tile_pool(name="t1p", bufs=1))
        opool = ctx.enter_context(tc.tile_pool(name="outp", bufs=4))

        s_all = cpool.tile([128, P], BF16, name="s_all")
        nc.sync.dma_start(out=s_all[:, :], in_=s_in[:, :])

        for r in range(reps):
          for t in range(NT):
            rows = slice(t * 128, (t + 1) * 128)
            xx = xpool.tile([128, 512], BF16, tag="xx", name=f"xx{r}_{t}",
                            bufs=2)
            nc.sync.dma_start(out=xx[:, 0:NCOL], in_=x_in[rows, :])
            nc.sync.dma_start(out=xx[:, NCOL:512], in_=x_in[rows, :])
            xxo = xpool.tile([128, 512], BF16, tag="xxo", name=f"xo{r}_{t}",
                             bufs=2)
            nc.sync.dma_start(out=xxo[:, 0:NCOL - 1], in_=x_in[rows, 1:NCOL])
            nc.sync.dma_start(out=xxo[:, NCOL - 1:2 * NCOL - 1],
                              in_=x_in[rows, :])
            t1 = tpool.tile([128, P], BF16, tag="t1", name=f"t1{r}_{t}",
                            bufs=1)
            if variant == "b3d":
                _pass1_b3d(nc, t1, xx, xxo)
            else:
                _pass1_per_d(nc, t1, xx, xxo)
            for k in range(CHUNKS):
                cs = slice(k * CH, (k + 1) * CH)
                o = opool.tile([128, CH], BF16, tag="o",
                               name=f"o{r}_{t}_{k}", bufs=4)
                nc.vector.tensor_mul(o[:, :], t1[:, cs], s_all[:, cs])
                nc.sync.dma_start(out=out_t[rows, cs], in_=o[:, :])

    nc.compile()
    return nc


class _Runner:
    def __init__(self, nc, n_cores=NCORES):
        install_neuronx_cc_hook()
        self.nc = nc
        self.n_cores = n_cores
        partition_name = (
            nc.partition_id_tensor.name if nc.partition_id_tensor else None
        )
        in_names, out_names, out_avals, zero_outs = [], [], [], []
        for alloc in nc.m.functions[0].allocations:
            if not isinstance(alloc, mybir.MemoryLocationSet):
                continue
            name = alloc.memorylocations[0].name
            if alloc.kind == "ExternalInput":
                if name != partition_name:
                    in_names.append(name)
            elif alloc.kind == "ExternalOutput":
                shape = tuple(alloc.tensor_shape)
                dtype = mybir.dt.np(alloc.dtype)
                out_avals.append(jax.core.ShapedArray(shape, dtype))
                zero_outs.append(np.zeros(shape, dtype))
                out_names.append(name)
        self.n_params = len(in_names)
        self.param_names = list(in_names)
        self.out_names = out_names
        self.out_avals = out_avals
        self.zero_outs = zero_outs
        all_in = in_names + out_names
        if partition_name is not None:
            all_in.append(partition_name)

        def _body(*args):
            operands = list(args)
            if partition_name is not None:
                operands.append(partition_id_tensor())
            return tuple(_bass_exec_p.bind(
                *operands,
                out_avals=tuple(out_avals),
                in_names=tuple(all_in),
                out_names=tuple(out_names),
                lowering_input_output_aliases=(),
                sim_require_finite=False,
                sim_require_nnan=False,
                nc=nc,
            ))

        if os.environ.get("K4_SIM"):
            devices = jax.devices("cpu")[:n_cores]
        else:
            devices = jax.devices()[:n_cores]
        mesh = Mesh(np.asarray(devices), ("core",))
        n_outs = len(out_names)
        in_specs = (PartitionSpec("core"),) * (self.n_params + n_outs)
        out_specs = (PartitionSpec("core"),) * n_outs
        self.fn = jax.jit(
            shard_map(_body, mesh=mesh, in_specs=in_specs,
                      out_specs=out_specs, check_rep=False),
            keep_unused=True,
        )

    def run_concat(self, concat_in):
        concat_zeros = [
            np.zeros((self.n_cores * z.shape[0], *z.shape[1:]), z.dtype)
            for z in self.zero_outs
        ]
        outs = self.fn(*concat_in, *concat_zeros)
        return [np.asarray(o) for o in outs]


_CACHE = {}


def _get_runner(reps=1, variant=VARIANT):
    key = (reps, variant)
    if key not in _CACHE:
        _CACHE[key] = _Runner(_build_nc(reps, variant))
    return _CACHE[key]


def _host_prep(x, latent_emb):
    x = np.asarray(x, np.float32)
    L = np.asarray(latent_emb, np.float32)
    s = (L @ L.T).astype(np.float32)

    # wrapped-diagonal S table
    sw = np.empty(P, np.float32)
    ii = np.arange(NCOL)
    for d in range(1, 128):
        sw[(d - 1) * NCOL:d * NCOL] = s[ii, (ii + d) % NCOL]
    sw[127 * NCOL:] = s[np.arange(128), np.arange(128) + 128]
    s_bcast = np.broadcast_to(sw.astype(BF16NP), (128, P)).copy()

    xb = np.ascontiguousarray(x.astype(BF16NP))         # [4096, 256]
    return xb, s_bcast


def _concat_inputs(runner, xb, s_bcast):
    concat_in = []
    for name in runner.param_names:
        if name == "x":
            concat_in.append(xb)
        elif name == "sw":
            concat_in.append(np.concatenate([s_bcast] * NCORES, axis=0))
        else:
            raise KeyError(name)
    return concat_in


_IDX = None


def _pair_index():
    global _IDX
    if _IDX is None:
        iu, ju = np.triu_indices(NCOL, k=1)
        d = ju - iu
        _IDX = np.where(
            d <= 127, (d - 1) * NCOL + iu,
            np.where(d == 128, 127 * NCOL + iu, (NCOL - d - 1) * NCOL + ju),
        ).astype(np.int64)
    return _IDX


def kernel(x, latent_emb):
    xb, s_bcast = _host_prep(x, latent_emb)
    runner = _get_runner()
    outs = runner.run_concat(_concat_inputs(runner, xb, s_bcast))
    dev = outs[runner.out_names.index("out")]     # [4096, P] bf16
    return dev[:, _pair_index()].astype(np.float32)
